# revision 1
# baseline (speedup 1.0000x reference)
"""ARAP loss kernel for Trainium2 (8 NeuronCores, SPMD, no collectives).

Math: for each batch b,
    out[b] = sum_{i,j} L[i,j] * |P[b,i,j]| / n_edges
where
    P[b,i,j] = c[b,i] + a[b,j] - 2*x[b,i]@xsub[b,j] + 2*dx[b,i]@dxsub[b,j]
    xsub = L @ x,  dxsub = L @ dx          (L symmetric {0,1})
    c[b,i] = |x[b,i]|^2 - |dx[b,i]|^2     (folded into the contraction
    a[b,j] = |xsub[b,j]|^2 - |dxsub[b,j]|^2    via x^2/dx^2 weight rows)

Sharding: column shard. Core c owns j in Jc (NV/8 = 512 columns). Its
single 4MB bf16 slice L[:, Jc] (resident in SBUF) serves both uses, via
symmetry:
  - pass 1: sub[Jc, d] = sum_m L[m, Jc] * V[m, d]   (PE, contraction on m)
  - pass 2: mask tiles L[i-chunk, Jc]
All matmuls run in bf16 (fp32 PE matmuls measured 5.7x slower). Precision
is retained by hi/lo bf16 splitting of V in pass 1 (two accumulating
matmuls) and of the dominant a[j] row of the moving operand in pass 2
(extra contraction row), with fp32 PSUM accumulation everywhere.

P is a rank-14 PE matmul per (b, i-chunk) tile, 4 i-chunks per PSUM group
(4 banks). Per group, either ACT extracts |P| to SBUF bf16 and a DVE
scalar_tensor_tensor multiplies by the L mask with a fused accumulated
row-sum, or (to balance engines) DVE mask-multiplies from PSUM and an
absolute-value tensor_reduce accumulates. Host only slices / reshapes /
casts inputs and sums the 8 partial outputs.

Hardware constraints honored: engine APs start at 32-aligned partitions
(pass-1 output packs b0@0, b1@32, n_edges@64 in one PSUM tile; DMA does
shifted placements), tensor_tensor_reduce avoided (faults on hw), STT
accum_out accumulates so acc is zeroed first.
"""

import sys

for _p in ("/opt/trn_rl_repo",):
    if _p not in sys.path:
        sys.path.insert(0, _p)

import contextlib

import numpy as np
import ml_dtypes

import concourse.bacc as bacc
import concourse.mybir as mybir
from concourse.tile import TileContext
from concourse import bass_utils

NV = 4096
B = 2
N_CORES = 8
JSH = NV // N_CORES          # 512 columns per core
NMC = NV // 128              # 32 chunks of 128 rows
NG = NMC // 4                # 8 PSUM groups of 4 chunks per batch
F32 = mybir.dt.float32
BF16 = mybir.dt.bfloat16
AF = mybir.ActivationFunctionType
ALU = mybir.AluOpType

# groups (flat index b*NG+g) routed to the DVE-only pipeline
ROUTE_B = frozenset({5, 10, 15})

_cached_nc = None


def _build_nc(route_b=ROUTE_B, repeat=1, ablate=(), scp_bufs=3, pm_bufs=2, dma_split=False, rowtile=False):
    nc = bacc.Bacc("TRN2", target_bir_lowering=False, debug=False)

    lcolb = nc.dram_tensor("lcolb", [NV, JSH], BF16, kind="ExternalInput")
    vthi = nc.dram_tensor("vthi", [128, NMC, 13], BF16, kind="ExternalInput")
    vtlo = nc.dram_tensor("vtlo", [128, NMC, 13], BF16, kind="ExternalInput")
    wtb = nc.dram_tensor("wtb", [B, 8, NV], BF16, kind="ExternalInput")
    cvec = nc.dram_tensor("cvec", [6, 2], F32, kind="ExternalInput")
    rconst = nc.dram_tensor("rconst", [6, JSH], F32, kind="ExternalInput")
    out = nc.dram_tensor("out", [1, 4], F32, kind="ExternalOutput")

    with TileContext(nc) as tc:
        with tc.tile_pool(name="res", bufs=1) as res:
            ltb = res.tile([128, NMC, JSH], BF16)   # resident L[:, Jc] bf16
            vh = res.tile([128, NMC, 65], BF16)     # V hi (b0@0,b1@32,one@64)
            vl = res.tile([128, NMC, 65], BF16)     # V lo
            wfb = res.tile([14, B, NV], BF16)       # x,dx,1,1,x^2,dx^2
            sqsb = res.tile([6, NV], BF16)          # squares staging (reused)
            Rm = res.tile([14, B, JSH], F32)        # moving operand (f32)
            Rb = res.tile([14, B, JSH], BF16)       # bf16 cast of Rm
            txdx = res.tile([38, B, JSH], F32)      # scaled sub staging
            s2p = res.tile([38, JSH], F32)          # sub squares (padded)
            ta0 = res.tile([1, JSH], F32)           # a_b staging
            ta1 = res.tile([1, JSH], F32)
            tah = res.tile([1, JSH], BF16)          # bf16(a)
            tah32 = res.tile([1, JSH], F32)
            talo = res.tile([1, JSH], F32)          # a - bf16(a)
            scl = res.tile([38, 1], F32)            # +-2 rows @0 and @32
            svec = res.tile([38, 1], F32)           # +-1 rows @0 and @32
            acc = res.tile([128, B * NG], F32)      # per-group partial sums
            ones128 = res.tile([128, 1], F32)
            red = res.tile([128, 2], F32)
            fin = res.tile([1, 4], F32)

            loop_ctx = (
                tc.For_i(0, repeat, 1) if repeat > 1
                else contextlib.nullcontext()
            )
            with loop_ctx:
                # ---- input DMAs ----
                lgrp = lcolb.rearrange("(g c p) j -> g p c j", c=4, p=128)
                for g in range(NMC // 4):
                    deng = nc.scalar if (dma_split and g % 2) else nc.sync
                    deng.dma_start(
                        out=ltb[:, 4 * g:4 * g + 4, :], in_=lgrp[g]
                    )
                nc.vector.memset(vh[:, :, :], 0.0)
                nc.vector.memset(vl[:, :, :], 0.0)
                nc.sync.dma_start(out=vh[:, :, 0:6], in_=vthi[:, :, 0:6])
                nc.sync.dma_start(out=vh[:, :, 32:38], in_=vthi[:, :, 6:12])
                nc.sync.dma_start(out=vh[:, :, 64:65], in_=vthi[:, :, 12:13])
                nc.sync.dma_start(out=vl[:, :, 0:6], in_=vtlo[:, :, 0:6])
                nc.sync.dma_start(out=vl[:, :, 32:38], in_=vtlo[:, :, 6:12])
                for b in range(B):
                    nc.sync.dma_start(out=wfb[0:8, b, :], in_=wtb[b])

                # ---- constants from host ----
                nc.vector.memset(ones128[:, :], 1.0)
                nc.vector.memset(acc[:, :], 0.0)
                nc.sync.dma_start(out=scl[0:6, :], in_=cvec[:, 0:1])
                nc.sync.dma_start(out=scl[32:38, :], in_=cvec[:, 0:1])
                nc.sync.dma_start(out=svec[0:6, :], in_=cvec[:, 1:2])
                nc.sync.dma_start(out=svec[32:38, :], in_=cvec[:, 1:2])
                for b in range(B):
                    nc.sync.dma_start(out=Rm[8:14, b, :], in_=rconst[:, :])

                if "dmaonly" in ablate:
                    nc.vector.memset(fin[:, :], 1.0)
                    nc.gpsimd.dma_start(out=out[:, :], in_=fin[:, :])

                # ---- weight squares: wfb[8:14] = (x, dx)^2 ----
                for b in range(B) if "dmaonly" not in ablate else []:
                    nc.scalar.activation(sqsb[0:6, :], wfb[0:6, b, :],
                                         AF.Square)
                    nc.gpsimd.dma_start(out=wfb[8:14, b, :], in_=sqsb[0:6, :])

                with tc.tile_pool(name="ph", bufs=1, space="PSUM") as ph:
                    sub = ph.tile([65, JSH], F32)   # b0@0..5,b1@32..37,ne@64
                    apb0 = ph.tile([1, JSH], F32)
                    apb1 = ph.tile([1, JSH], F32)
                    apbs = [apb0, apb1]

                    # ---- pass 1: sub + n_edges, streaming L (bf16) ----
                    for mc in range(NMC) if "dmaonly" not in ablate else []:
                        nc.tensor.matmul(
                            sub[:, :], lhsT=vh[:, mc, :], rhs=ltb[:, mc, :],
                            start=(mc == 0), stop=False,
                        )
                        nc.tensor.matmul(
                            sub[:, :], lhsT=vl[:, mc, :], rhs=ltb[:, mc, :],
                            start=False, stop=(mc == NMC - 1),
                        )

                    if "dmaonly" not in ablate:
                        nc.vector.tensor_reduce(
                            fin[:, 2:3], sub[64:65, :],
                            axis=mybir.AxisListType.X, op=ALU.add,
                        )
                    nc.vector.memset(fin[:, 3:4], 0.0)

                    # ---- build R per batch ----
                    ta = [ta0, ta1]
                    for b in range(B) if "dmaonly" not in ablate else []:
                        lo = 32 * b      # b0 rows @0..5, b1 rows @32..37
                        sb6 = sub[lo:lo + 6, :]
                        # rows 0..5: (-2*xsub, +2*dxsub) per-partition scale
                        nc.scalar.activation(
                            txdx[lo:lo + 6, b, :], sb6, AF.Copy,
                            scale=scl[lo:lo + 6, :],
                        )
                        nc.gpsimd.dma_start(
                            out=Rm[0:6, b, :], in_=txdx[lo:lo + 6, b, :]
                        )
                        # rows 6,7: a_b = sum_d xsub^2 - dxsub^2, hi/lo split
                        nc.scalar.activation(s2p[lo:lo + 6, :], sb6, AF.Square)
                        nc.tensor.matmul(
                            apbs[b][:, :], lhsT=svec[lo:lo + 6, :],
                            rhs=s2p[lo:lo + 6, :], start=True, stop=True,
                        )
                        nc.scalar.copy(ta[b][:, :], apbs[b][:, :])
                        nc.vector.tensor_copy(out=tah[:, :], in_=ta[b][:, :])
                        nc.vector.tensor_copy(out=tah32[:, :], in_=tah[:, :])
                        nc.vector.tensor_tensor(
                            out=talo[:, :], in0=ta[b][:, :], in1=tah32[:, :],
                            op=ALU.subtract,
                        )
                        nc.gpsimd.dma_start(out=Rm[6:7, b, :], in_=ta[b][:, :])
                        nc.gpsimd.dma_start(out=Rm[7:8, b, :], in_=talo[:, :])

                for b in range(B) if "dmaonly" not in ablate else []:
                    nc.vector.tensor_copy(out=Rb[0:14, b, :],
                                            in_=Rm[0:14, b, :])

                # ---- main: P in 4-bank PSUM groups + fused row-sums ----
                with (
                    tc.tile_pool(name="pm", bufs=pm_bufs, space="PSUM") as pm,
                    tc.tile_pool(name="scp", bufs=scp_bufs) as scp,
                ):
                    for b in range(B) if "dmaonly" not in ablate else []:
                        for g in range(NG):
                            pt4 = pm.tile([128, 4, JSH], F32, tag="pt",
                                          name="pt")
                            for k in range(4):
                                if "onemm" in ablate and k > 0:
                                    continue
                                ic = 4 * g + k
                                lo = 0
                                nc.tensor.matmul(
                                    pt4[:, k, :],
                                    lhsT=wfb[lo:lo + 14, b,
                                             ic * 128:(ic + 1) * 128],
                                    rhs=Rb[lo:lo + 14, b, :],
                                    start=True, stop=True,
                                )
                            flat = b * NG + g
                            sl4 = slice(4 * g, 4 * g + 4)
                            if "noextract" in ablate:
                                continue
                            if flat not in route_b:
                                # ACT abs-extract; DVE masked mult-accum
                                ab4 = scp.tile([128, 4, JSH], BF16, tag="ab",
                                               name="ab")
                                nc.scalar.activation(
                                    ab4[:, :, :], pt4[:, :, :], AF.Abs
                                )
                                sct = scp.tile([128, 4, JSH], BF16,
                                               tag="sct", name="sct")
                                nc.vector.scalar_tensor_tensor(
                                    out=sct[:, :, :],
                                    in0=ab4[:, :, :],
                                    scalar=1.0,
                                    in1=ltb[:, sl4, :],
                                    op0=ALU.mult,
                                    op1=ALU.mult,
                                    accum_out=acc[:, flat:flat + 1],
                                )
                            else:
                                # DVE mask-extract; DVE abs-reduce
                                sct = scp.tile([128, 4, JSH], BF16,
                                               tag="sct", name="sct")
                                nc.vector.tensor_tensor(
                                    out=sct[:, :, :], in0=pt4[:, :, :],
                                    in1=ltb[:, sl4, :], op=ALU.mult,
                                )
                                nc.vector.tensor_reduce(
                                    acc[:, flat:flat + 1], sct[:, :, :],
                                    axis=mybir.AxisListType.XY, op=ALU.add,
                                    apply_absolute_value=True,
                                )

                with tc.tile_pool(name="pf", bufs=1, space="PSUM") as pf:
                    if "dmaonly" in ablate:
                        pf.tile([1, 2], F32, name="dummy")
                    for b in range(B) if "dmaonly" not in ablate else []:
                        nc.vector.tensor_reduce(
                            red[:, b:b + 1], acc[:, b * NG:(b + 1) * NG],
                            axis=mybir.AxisListType.X, op=ALU.add,
                        )
                    if "dmaonly" not in ablate:
                        fp = pf.tile([1, 2], F32)
                        nc.tensor.matmul(
                            fp[:, :], lhsT=ones128[:, :], rhs=red[:, :],
                            start=True, stop=True,
                        )
                        nc.scalar.copy(fin[:, 0:2], fp[:, :])
                        nc.gpsimd.dma_start(out=out[:, :], in_=fin[:, :])

    nc.compile()
    return nc


def _get_nc():
    global _cached_nc
    if _cached_nc is None:
        _cached_nc = _build_nc()
    return _cached_nc


def _prep_inputs(dx, x, laplacian):
    x = np.asarray(x, dtype=np.float32)
    dx = np.asarray(dx, dtype=np.float32)
    L = np.asarray(laplacian, dtype=np.float32)

    vin = np.zeros((NV, 13), dtype=np.float32)
    vin[:, 0:3] = x[0]
    vin[:, 3:6] = dx[0]
    vin[:, 6:9] = x[1]
    vin[:, 9:12] = dx[1]
    vin[:, 12] = 1.0
    vhi = vin.astype(ml_dtypes.bfloat16)
    vlo = (vin - vhi.astype(np.float32)).astype(ml_dtypes.bfloat16)
    # [m, d] -> [p, chunk, d] so each partition's DMA data is contiguous
    vthi = np.ascontiguousarray(vhi.reshape(NMC, 128, 13).transpose(1, 0, 2))
    vtlo = np.ascontiguousarray(vlo.reshape(NMC, 128, 13).transpose(1, 0, 2))

    wt = np.empty((B, 8, NV), dtype=np.float32)
    for b in range(B):
        wt[b, 0:3] = x[b].T
        wt[b, 3:6] = dx[b].T
        wt[b, 6] = 1.0
        wt[b, 7] = 1.0
    wtb = wt.astype(ml_dtypes.bfloat16)

    cvec = np.empty((6, 2), dtype=np.float32)
    cvec[0:3, 0] = -2.0
    cvec[3:6, 0] = 2.0
    cvec[0:3, 1] = 1.0
    cvec[3:6, 1] = -1.0

    rconst = np.empty((6, JSH), dtype=np.float32)
    rconst[0:3] = 1.0
    rconst[3:6] = -1.0

    in_maps = []
    for c in range(N_CORES):
        lcolb = np.ascontiguousarray(
            L[:, c * JSH:(c + 1) * JSH]
        ).astype(ml_dtypes.bfloat16)
        in_maps.append(
            {"lcolb": lcolb, "vthi": vthi, "vtlo": vtlo, "wtb": wtb,
             "cvec": cvec, "rconst": rconst}
        )
    return in_maps


def run(dx, x, laplacian, trace=False):
    nc = _get_nc()
    in_maps = _prep_inputs(dx, x, laplacian)
    res = bass_utils.run_bass_kernel_spmd(
        nc, in_maps, core_ids=list(range(N_CORES)), trace=trace
    )
    parts = np.stack([res.results[c]["out"][0] for c in range(N_CORES)])
    sums = parts[:, 0:2].sum(axis=0)
    n_edges = parts[:, 2].sum()
    outv = (sums / n_edges).astype(np.float32)
    return outv, res


def kernel(dx, x, laplacian):
    outv, _ = run(dx, x, laplacian, trace=False)
    return outv

